# revision 54
# baseline (speedup 1.0000x reference)
"""MoE gating network (GatingNetwork) on 8 TRN2 NeuronCores.

Data-parallel: the token dim of x is sharded across 8 cores; the tiny router
weights are replicated. Per core (4096 tokens), all matmul operands are fp16
(11 effective mantissa bits -- same error class as a float32r kernel at half
the HBM traffic and 1 PE cycle/row):

  xT (host-pre-transposed fp16) --matmul--> h^T [hidden, tok] in PSUM (fp32)
  --ReLU+b1 (ACT)--> hrelu^T fp16 in SBUF
  --fp16 matmul--> logits in PSUM --DVE +b2, top-8 values / indices written
  straight into a [128, ntiles, 16] i32 result image (vals f32 | idx u32)
  that leaves in two DMAs (bulk early, last chunks at the end). For the two
  final (tail-critical) chunks b2 is instead folded into the logits matmul
  via a ones-row matmul so the DVE chain reads PSUM directly.

Gates (softmax over the top-2 logits) are computed on the host from the
exported top-3 values. Tokens whose top-3 logit margins fall below tau are
recomputed exactly on the host; for every kept token the exported margin
exceeds twice the device's logit error bound, so its ranking provably
matches an exact computation.

Timeline structure (all tuned against the TRN2 cost model):
  - a PE warmup matmul chain primes the p-state ramp before real work
  - x streams in token-chunks (small ones first so the PE starts early)
  - mm2/topk for chunk c-1 are emitted between mm1(c) and mm1(c+1) so the
    ACT->mm2 dependency never stalls the PE
  - the bulk output DMA is issued before the last chunk computes; only the
    final two chunks' tiles ride the end-of-program DMA chain
"""
import numpy as np
import concourse.bass as bass
import concourse.mybir as mybir
from concourse.tile import TileContext
from concourse.bass_utils import run_bass_kernel_spmd

N_TOKENS = 32768
INPUT_DIM = 1024
HIDDEN_DIM = 256
NUM_EXPERTS = 64
N_CORES = 8
NT = N_TOKENS // N_CORES        # tokens per core
NTILES = NT // 128
FIXUP_TAU = 8e-3

F32 = mybir.dt.float32
F16 = mybir.dt.float16
U32 = mybir.dt.uint32
I32 = mybir.dt.int32
AF = mybir.ActivationFunctionType

# token chunk lengths processed as one pipeline unit (must sum to NT; each a
# multiple of 128). Small leading chunks let the PE start while DMA streams;
# small trailing chunks shrink the post-PE topk tail.
SCHEDULE = [256] * 15 + [128] * 2
# PE warmup matmul row counts (reads the pre-barrier const-0 SBUF tensor, so
# the first warmup issues right after the PE preamble and pe_busy_start is
# primed ~3.5us before the first real matmul; output never read)
WARM_ROWS = [128] * 8



def _split_excess_waits(nc, max_waits=1):
    """walrus in this toolchain accepts at most one sem wait per
    instruction; hoist extras onto preceding NoOps on the same engine."""
    n_new = 0
    for fn in nc.m.functions:
        for bb in fn.blocks:
            new_insts = []
            for inst in bb.instructions:
                si = getattr(inst, "sync_info", None)
                waits = list(si.on_wait) if si is not None and si.on_wait else []
                if len(waits) > max_waits:
                    excess = waits[:-max_waits]
                    si.on_wait = waits[-max_waits:]
                    for j in range(0, len(excess), max_waits):
                        n_new += 1
                        new_insts.append(mybir.InstNoOp(
                            name=f"wait-split-{n_new}",
                            engine=inst.engine,
                            ins=[], outs=[],
                            sync_info=mybir.SyncInfo(
                                on_wait=excess[j:j + max_waits], on_update=[]),
                        ))
                new_insts.append(inst)
            bb.instructions[:] = new_insts
    return n_new


def _strip_trailing_barrier(nc):
    """TileContext exit emits two all-engine drain+barrier rounds; the second
    (after the Pool end-of-program ISA marker) is redundant -- every engine
    already quiesced in round one. Drop it if the expected pattern is found."""
    bb = nc.m.functions[0].blocks[-1]
    insts = bb.instructions
    isa_pos = [i for i, inst in enumerate(insts)
               if type(inst).__name__ == "InstISA"]
    if not isa_pos:
        return False
    cut = isa_pos[-1] + 1
    tail = insts[cut:]
    if tail and all(type(i).__name__ in ("InstDrain", "InstEventSemaphore")
                    for i in tail):
        del insts[cut:]
        return True
    return False


def build_kernel(nt=NT, x_bufs=5, schedule=None, warm_rows=None,
                 lg_bufs=3, split_back=2, n_split=2):
    """Build the SPMD program one core runs on its `nt`-token shard."""
    schedule = list(SCHEDULE) if schedule is None else list(schedule)
    warm_rows = list(WARM_ROWS) if warm_rows is None else list(warm_rows)
    nchunks = len(schedule)
    assert sum(schedule) == nt and all(L % 128 == 0 for L in schedule)
    ntiles = nt // 128

    nc = bass.Bass(target_bir_lowering=False)

    xT = nc.dram_tensor("xT", [INPUT_DIM, nt], F16, kind="ExternalInput")
    w1a = nc.dram_tensor("w1a", [128, 8 * 128], F16, kind="ExternalInput")
    w1b = nc.dram_tensor("w1b", [128, 8 * 128], F16, kind="ExternalInput")
    # cblob: cols 0:2 = b1 halves (f32 bits), 2:66 = w2 image (f16 bits),
    # 66:130 = b2 broadcast to every partition (f32 bits), partition 0
    # cols 130:162 = b2 as bf16 for the tail ones-row matmul
    cblob = nc.dram_tensor("cblob", [128, 162], I32, kind="ExternalInput")
    out = nc.dram_tensor("out", [128, ntiles * 16], I32, kind="ExternalOutput")

    with TileContext(nc) as tc:
        with (
            tc.tile_pool(name="const", bufs=1) as cpool,
            tc.tile_pool(name="xin", bufs=x_bufs) as xpool,
            tc.tile_pool(name="hrelu", bufs=2) as hpool,
            tc.tile_pool(name="lsb", bufs=2) as lspool,
            tc.tile_pool(name="res", bufs=1) as rpool,
            tc.tile_pool(name="hps", bufs=2, space="PSUM") as hpsum,
            tc.tile_pool(name="lps", bufs=lg_bufs, space="PSUM") as lpsum,
            tc.tile_pool(name="wps", bufs=1, space="PSUM") as wpsum,
        ):
            # ---- PE warmup: prime the p-state ramp before real work ----
            # operands are the pre-barrier const-0 SBUF tensor, so nothing
            # gates these but the PE preamble itself
            zcol = nc.const_aps.tensor(0.0, (128, 1), F32)
            wp = wpsum.tile([1, 512], F32, tag="wp")
            for r in warm_rows:
                nc.tensor.matmul(wp[0:1, 0:r], zcol[:, 0:1],
                                 nc.const_aps.tensor(0.0, (128, r), F32),
                                 start=True, stop=True)

            # ---- constants / inputs (issue order = stream order) ----
            w1a_sb = cpool.tile([128, 8, 128], F16, tag="w1a")
            w1b_sb = cpool.tile([128, 8, 128], F16, tag="w1b")
            cb = cpool.tile([128, 162], I32, tag="cb")
            b1_sb = cb[:, 0:2].bitcast(F32)             # [128, 2]
            w2v = cb[:, 2:66].bitcast(F16)              # [128, 128]
            b2_sb = cb[:, 66:130].bitcast(F32)          # [128, 64] broadcast
            b2row = cb[0:1, 130:162].bitcast(mybir.dt.bfloat16)  # [1, 64]
            ones_row = nc.const_aps.tensor(1.0, (1, 128), mybir.dt.bfloat16)

            # ---- result image: max/max_index write straight into it ----
            # per tile: cols 0:8 top-8 values (f32 bits), 8:16 top-8 indices
            packed = rpool.tile([128, ntiles, 16], I32, tag="packed")

            offs = [sum(schedule[:i]) for i in range(len(schedule))]
            xts, hrs, lps = {}, {}, {}

            def load_chunk(ci, half=None):
                L = schedule[ci]
                if half is None or half == 0:
                    xt = xpool.tile([128, 8, L], F16, tag="xt", name=f"xt{ci}")
                    xts[ci] = xt
                xt = xts[ci]
                ks = slice(0, 8) if half is None else slice(4 * half, 4 * half + 4)
                nk = 8 if half is None else 4
                nc.sync.dma_start(
                    xt[:, ks, :],
                    bass.AP(xT, offs[ci] + (0 if not half else 4 * 128 * nt),
                            [[nt, 128], [128 * nt, nk], [1, L]]))

            def mm1(ci):
                L = schedule[ci]
                xt = xts[ci]
                hr = []
                for m, w_sb in ((0, w1a_sb), (1, w1b_sb)):
                    hp = hpsum.tile([128, L], F32, tag=f"h{m}",
                                    name=f"hp{m}_{ci}", padded_shape=[128, 512])
                    for k in range(8):
                        nc.tensor.matmul(
                            hp[:, :], w_sb[:, k, :], xt[:, k, :],
                            start=(k == 0), stop=(k == 7))
                    hrm = hpool.tile([128, L], F16, tag=f"hr{m}",
                                     name=f"hr{m}_{ci}", padded_shape=[128, 512])
                    nc.scalar.activation(hrm[:, :], hp[:, :], AF.Relu,
                                         bias=b1_sb[:, m:m + 1])
                    hr.append(hrm)
                hrs[ci] = hr

            def mm2(ci):
                L = schedule[ci]
                ns = L // 128
                hr = hrs[ci]
                lp = lpsum.tile([128, ns, NUM_EXPERTS], F32, tag="lg",
                                name=f"lp{ci}",
                                padded_shape=[128, 4, NUM_EXPERTS])
                # in the tail, fold b2 in via a ones-row matmul (27ns on the
                # PE) so the tail-critical DVE chain skips the +b2 add and
                # reads logits straight from PSUM
                fold_b2 = ci >= len(schedule) - 2
                for s in range(ns):
                    if fold_b2:
                        nc.tensor.matmul(lp[:, s, :], ones_row[0:1, :],
                                         b2row[0:1, :], start=True, stop=False)
                    nc.tensor.matmul(lp[:, s, :],
                                     hr[0][:, s * 128:(s + 1) * 128],
                                     w2v[:, 0:64], start=not fold_b2,
                                     stop=False)
                    nc.tensor.matmul(lp[:, s, :],
                                     hr[1][:, s * 128:(s + 1) * 128],
                                     w2v[:, 64:128], start=False, stop=True)
                lps[ci] = lp

            def topk(ci):
                L = schedule[ci]
                ns = L // 128
                t0 = offs[ci] // 128
                lp = lps[ci]
                if ci >= len(schedule) - 2:
                    for s in range(ns):
                        t = t0 + s
                        nc.vector.max(out=packed[:, t, 0:8].bitcast(F32),
                                      in_=lp[:, s, :])
                        nc.vector.max_index(
                            out=packed[:, t, 8:16].bitcast(U32),
                            in_max=packed[:, t, 0:8].bitcast(F32),
                            in_values=lp[:, s, :])
                    return
                lg = lspool.tile([128, ns, NUM_EXPERTS], F32, tag="lsb",
                                 name=f"lg{ci}",
                                 padded_shape=[128, 4, NUM_EXPERTS])
                for s in range(ns):
                    t = t0 + s
                    nc.vector.tensor_add(lg[:, s, :], lp[:, s, :], b2_sb)
                    nc.vector.max(out=packed[:, t, 0:8].bitcast(F32),
                                  in_=lg[:, s, :])
                    nc.vector.max_index(out=packed[:, t, 8:16].bitcast(U32),
                                        in_max=packed[:, t, 0:8].bitcast(F32),
                                        in_values=lg[:, s, :])

            # DMA stream order: w1a, x0 (two halves), w1b, x1, cblob, x2...
            load_w1a = lambda: nc.sync.dma_start(
                w1a_sb[:, :, :],
                bass.AP(w1a, 0, [[8 * 128, 128], [128, 8], [1, 128]]))
            load_w1b = lambda: nc.sync.dma_start(
                w1b_sb[:, :, :],
                bass.AP(w1b, 0, [[8 * 128, 128], [128, 8], [1, 128]]))
            load_cb = lambda: nc.sync.dma_start(
                cb[:, :], bass.AP(cblob, 0, [[162, 128], [1, 162]]))

            # first chunks stream in half-k pieces so the PE can start on
            # k=0..3 while k=4..7 is still in flight
            load_w1a()
            load_chunk(0, half=0)
            load_chunk(0, half=1)
            load_w1b()
            for ci in range(1, n_split):
                load_chunk(ci, half=0)
                load_chunk(ci, half=1)
            load_cb()
            for ci in range(n_split, min(n_split + 1, nchunks)):
                load_chunk(ci)

            t_split = offs[nchunks - split_back] // 128
            for ci in range(nchunks):
                if n_split < ci + 1 < nchunks and ci >= 1:
                    load_chunk(ci + 1)
                mm1(ci)
                if ci > 0:
                    mm2(ci - 1)
                    topk(ci - 1)
                if ci == nchunks - 1:
                    # bulk of the output: its DMA chain overlaps the tail
                    # compute (all x loads are already dispatched on SP)
                    nc.sync.dma_start(
                        bass.AP(out, 0, [[ntiles * 16, 128], [1, t_split * 16]]),
                        packed[:, 0:t_split, :])
            mm2(nchunks - 1)
            topk(nchunks - 1)

            nc.sync.dma_start(
                bass.AP(out, t_split * 16,
                        [[ntiles * 16, 128], [1, (ntiles - t_split) * 16]]),
                packed[:, t_split:ntiles, :])

    _split_excess_waits(nc)
    _strip_trailing_barrier(nc)
    return nc


def shard_inputs(x, w1, b1, w2, b2, n_cores=N_CORES):
    nt = x.shape[0] // n_cores
    w1T = np.ascontiguousarray(w1.T).astype(np.float16)        # [1024, 256]
    w1r = w1T.reshape(8, 128, HIDDEN_DIM)                      # [k, p, h]
    w1ai = np.ascontiguousarray(
        w1r[:, :, 0:128].transpose(1, 0, 2).reshape(128, 8 * 128))
    w1bi = np.ascontiguousarray(
        w1r[:, :, 128:256].transpose(1, 0, 2).reshape(128, 8 * 128))
    w2T = np.ascontiguousarray(w2.T).astype(np.float16)        # [256, 64]
    w2i = np.ascontiguousarray(
        w2T.reshape(2, 128, NUM_EXPERTS).transpose(1, 0, 2)
        .reshape(128, 2 * NUM_EXPERTS))                        # [128, 128] f16
    b1i = np.ascontiguousarray(b1.reshape(2, 128).T.astype(np.float32))
    cblob = np.zeros((128, 162), np.int32)
    cblob[:, 0:2] = b1i.view(np.int32)
    cblob[:, 2:66] = w2i.view(np.int32)
    cblob[:, 66:130] = np.broadcast_to(
        b2.astype(np.float32).view(np.int32), (128, 64))
    # b2 as bf16 (round-to-nearest-even) for the tail ones-row matmul
    b2u = b2.astype(np.float32).view(np.uint32)
    b2bf = ((b2u + 0x7FFF + ((b2u >> 16) & 1)) >> 16).astype(np.uint16)
    cblob[0, 130:162] = b2bf.view(np.int32)
    xT = np.ascontiguousarray(x.T.astype(np.float16))          # [1024, N]
    return [
        {"xT": np.ascontiguousarray(xT[:, c * nt:(c + 1) * nt]),
         "w1a": w1ai, "w1b": w1bi, "cblob": cblob}
        for c in range(n_cores)
    ]


def unshard_outputs(results, nt=NT):
    ntiles = nt // 128
    idxs, maxes = [], []
    for res in results:
        packed = res["out"].reshape(128, ntiles, 16)
        m = np.ascontiguousarray(packed[:, :, 0:3]).view(np.float32)
        i = packed[:, :, 8:10]
        maxes.append(m.transpose(1, 0, 2).reshape(nt, 3))
        idxs.append(i.transpose(1, 0, 2).reshape(nt, 2).astype(np.int32))
    return np.concatenate(idxs), np.concatenate(maxes)


def host_gates(maxes):
    """softmax over the top-2 logits, from the exported top-3 values."""
    d = (maxes[:, 1] - maxes[:, 0]).astype(np.float32)
    e = np.exp(d)
    g1 = 1.0 / (1.0 + e)
    return np.stack([g1, e * g1], axis=1).astype(np.float32)


def margin_fixup(idx, gates, maxes, x, w1, b1, w2, b2, tau=FIXUP_TAU):
    """Exactly recompute tokens whose device top-3 margins are below tau."""
    margin = np.minimum(maxes[:, 0] - maxes[:, 1], maxes[:, 1] - maxes[:, 2])
    bad = np.where(margin < tau)[0]
    if len(bad) == 0:
        return idx, gates, bad
    xb = x[bad].astype(np.float64)
    h = np.maximum(xb @ w1.astype(np.float64).T + b1.astype(np.float64), 0)
    logits = h @ w2.astype(np.float64).T + b2.astype(np.float64)
    order = np.argsort(-logits, axis=1)[:, :2]
    m = np.take_along_axis(logits, order, axis=1)
    e = np.exp(m - m[:, :1])
    g = (e / e.sum(axis=1, keepdims=True)).astype(np.float32)
    idx = idx.copy(); gates = gates.copy()
    idx[bad] = order.astype(np.int32)
    gates[bad] = g
    return idx, gates, bad


_NC_CACHE = None


def _get_nc():
    global _NC_CACHE
    if _NC_CACHE is None:
        _NC_CACHE = build_kernel()
    return _NC_CACHE


def run_on_device(x, w1, b1, w2, b2, **spmd_kwargs):
    """Run the Bass kernel on the 8 cores; returns (idx, maxes) plus
    the raw BassKernelResults (for profiling)."""
    in_maps = shard_inputs(x, w1, b1, w2, b2)
    res = run_bass_kernel_spmd(_get_nc(), in_maps, list(range(N_CORES)),
                               **spmd_kwargs)
    idx, maxes = unshard_outputs(res.results)
    return idx, maxes, res


def kernel(x, w1, b1, w2, b2):
    x = np.asarray(x, dtype=np.float32)
    w1 = np.asarray(w1, dtype=np.float32)
    b1 = np.asarray(b1, dtype=np.float32)
    w2 = np.asarray(w2, dtype=np.float32)
    b2 = np.asarray(b2, dtype=np.float32)
    idx, maxes, _ = run_on_device(x, w1, b1, w2, b2)
    gates = host_gates(maxes)
    idx, gates, _ = margin_fixup(idx, gates, maxes, x, w1, b1, w2, b2)
    return idx.astype(np.int32), gates.astype(np.float32)


# revision 56
# speedup vs baseline: 1.0099x; 1.0099x over previous
"""MoE gating network (GatingNetwork) on 8 TRN2 NeuronCores.

Data-parallel: the token dim of x is sharded across 8 cores; the tiny router
weights are replicated. Per core (4096 tokens), all matmul operands are fp16
(11 effective mantissa bits -- same error class as a float32r kernel at half
the HBM traffic and 1 PE cycle/row):

  xT (host-pre-transposed fp16) --matmul--> h^T [hidden, tok] in PSUM (fp32)
  --ReLU+b1 (ACT)--> hrelu^T fp16 in SBUF
  --fp16 matmul--> logits in PSUM --DVE +b2, top-8 values / indices written
  straight into a [128, ntiles, 16] i32 result image (vals f32 | idx u32)
  that leaves in two DMAs (bulk early, last chunks at the end). For the two
  final (tail-critical) chunks b2 is instead folded into the logits matmul
  via a ones-row matmul so the DVE chain reads PSUM directly.

Gates (softmax over the top-2 logits) are computed on the host from the
exported top-3 values. Tokens whose top-3 logit margins fall below tau are
recomputed exactly on the host; for every kept token the exported margin
exceeds twice the device's logit error bound, so its ranking provably
matches an exact computation.

Timeline structure (all tuned against the TRN2 cost model):
  - a PE warmup matmul chain primes the p-state ramp before real work
  - x streams in token-chunks (small ones first so the PE starts early)
  - mm2/topk for chunk c-1 are emitted between mm1(c) and mm1(c+1) so the
    ACT->mm2 dependency never stalls the PE
  - the bulk output DMA is issued before the last chunk computes; only the
    final two chunks' tiles ride the end-of-program DMA chain
"""
import numpy as np
import concourse.bass as bass
import concourse.mybir as mybir
from concourse.tile import TileContext
from concourse.bass_utils import run_bass_kernel_spmd

N_TOKENS = 32768
INPUT_DIM = 1024
HIDDEN_DIM = 256
NUM_EXPERTS = 64
N_CORES = 8
NT = N_TOKENS // N_CORES        # tokens per core
NTILES = NT // 128
FIXUP_TAU = 8e-3

F32 = mybir.dt.float32
F16 = mybir.dt.float16
U32 = mybir.dt.uint32
I32 = mybir.dt.int32
AF = mybir.ActivationFunctionType

# token chunk lengths processed as one pipeline unit (must sum to NT; each a
# multiple of 128). Small leading chunks let the PE start while DMA streams;
# small trailing chunks shrink the post-PE topk tail.
SCHEDULE = [256] * 15 + [128] * 2
# PE warmup matmul row counts (reads the pre-barrier const-0 SBUF tensor, so
# the first warmup issues right after the PE preamble and pe_busy_start is
# primed ~3.5us before the first real matmul; output never read)
WARM_ROWS = [128] * 8



def _split_excess_waits(nc, max_waits=1):
    """walrus in this toolchain accepts at most one sem wait per
    instruction; hoist extras onto preceding NoOps on the same engine."""
    n_new = 0
    for fn in nc.m.functions:
        for bb in fn.blocks:
            new_insts = []
            for inst in bb.instructions:
                si = getattr(inst, "sync_info", None)
                waits = list(si.on_wait) if si is not None and si.on_wait else []
                if len(waits) > max_waits:
                    excess = waits[:-max_waits]
                    si.on_wait = waits[-max_waits:]
                    for j in range(0, len(excess), max_waits):
                        n_new += 1
                        new_insts.append(mybir.InstNoOp(
                            name=f"wait-split-{n_new}",
                            engine=inst.engine,
                            ins=[], outs=[],
                            sync_info=mybir.SyncInfo(
                                on_wait=excess[j:j + max_waits], on_update=[]),
                        ))
                new_insts.append(inst)
            bb.instructions[:] = new_insts
    return n_new


def _strip_trailing_barrier(nc):
    """TileContext exit emits two all-engine drain+barrier rounds; the second
    (after the Pool end-of-program ISA marker) is redundant -- every engine
    already quiesced in round one. Drop it if the expected pattern is found.
    Also drop round one's cross-engine EventSemaphores: each engine's Drain
    already waits out its own queues, so the extra all-to-all sem exchange
    only delays the end-of-program marker."""
    bb = nc.m.functions[0].blocks[-1]
    insts = bb.instructions
    isa_pos = [i for i, inst in enumerate(insts)
               if type(inst).__name__ == "InstISA"]
    if not isa_pos:
        return False
    cut = isa_pos[-1] + 1
    tail = insts[cut:]
    if tail and all(type(i).__name__ in ("InstDrain", "InstEventSemaphore")
                    for i in tail):
        del insts[cut:]
    keep = []
    for i, inst in enumerate(insts):
        if (i < isa_pos[-1]
                and type(inst).__name__ == "InstEventSemaphore"):
            continue
        keep.append(inst)
    insts[:] = keep
    return True


def _strip_unused_const_memsets(nc):
    """Bass init memsets four const-AP tensors on the Pool engine before the
    all-engine barrier; this kernel only reads const-float32-0.0 (PE warmup)
    and const-bfloat16-1.0 (the b2 ones-row). Dropping the two unused
    memsets shortens the Pool preamble, which gates the init barrier and
    with it the first DMA dispatch."""
    drop = {"const-float32-1.0", "const-uint8-127"}
    bb = nc.m.functions[0].blocks[0]
    keep = []
    for inst in bb.instructions:
        if type(inst).__name__ == "InstMemset":
            ba = getattr(inst.outs[0], "bass_ap", None)
            if ba is not None and getattr(ba, "name", None) in drop:
                continue
        keep.append(inst)
    n = len(bb.instructions) - len(keep)
    bb.instructions[:] = keep
    return n


def build_kernel(nt=NT, x_bufs=5, schedule=None, warm_rows=None,
                 lg_bufs=3, split_back=2, n_split=2):
    """Build the SPMD program one core runs on its `nt`-token shard."""
    schedule = list(SCHEDULE) if schedule is None else list(schedule)
    warm_rows = list(WARM_ROWS) if warm_rows is None else list(warm_rows)
    nchunks = len(schedule)
    assert sum(schedule) == nt and all(L % 128 == 0 for L in schedule)
    ntiles = nt // 128

    nc = bass.Bass(target_bir_lowering=False)

    xT = nc.dram_tensor("xT", [INPUT_DIM, nt], F16, kind="ExternalInput")
    w1a = nc.dram_tensor("w1a", [128, 8 * 128], F16, kind="ExternalInput")
    w1b = nc.dram_tensor("w1b", [128, 8 * 128], F16, kind="ExternalInput")
    # cblob: cols 0:2 = b1 halves (f32 bits), 2:66 = w2 image (f16 bits),
    # 66:130 = b2 broadcast to every partition (f32 bits), partition 0
    # cols 130:162 = b2 as bf16 for the tail ones-row matmul
    cblob = nc.dram_tensor("cblob", [128, 162], I32, kind="ExternalInput")
    out = nc.dram_tensor("out", [128, ntiles * 16], I32, kind="ExternalOutput")

    with TileContext(nc) as tc:
        with (
            tc.tile_pool(name="const", bufs=1) as cpool,
            tc.tile_pool(name="xin", bufs=x_bufs) as xpool,
            tc.tile_pool(name="hrelu", bufs=2) as hpool,
            tc.tile_pool(name="lsb", bufs=2) as lspool,
            tc.tile_pool(name="res", bufs=1) as rpool,
            tc.tile_pool(name="hps", bufs=2, space="PSUM") as hpsum,
            tc.tile_pool(name="lps", bufs=lg_bufs, space="PSUM") as lpsum,
            tc.tile_pool(name="wps", bufs=1, space="PSUM") as wpsum,
        ):
            # ---- PE warmup: prime the p-state ramp before real work ----
            # operands are the pre-barrier const-0 SBUF tensor, so nothing
            # gates these but the PE preamble itself
            zcol = nc.const_aps.tensor(0.0, (128, 1), F32)
            wp = wpsum.tile([1, 512], F32, tag="wp")
            for r in warm_rows:
                nc.tensor.matmul(wp[0:1, 0:r], zcol[:, 0:1],
                                 nc.const_aps.tensor(0.0, (128, r), F32),
                                 start=True, stop=True)

            # ---- constants / inputs (issue order = stream order) ----
            w1a_sb = cpool.tile([128, 8, 128], F16, tag="w1a")
            w1b_sb = cpool.tile([128, 8, 128], F16, tag="w1b")
            cb = cpool.tile([128, 162], I32, tag="cb")
            b1_sb = cb[:, 0:2].bitcast(F32)             # [128, 2]
            w2v = cb[:, 2:66].bitcast(F16)              # [128, 128]
            b2_sb = cb[:, 66:130].bitcast(F32)          # [128, 64] broadcast
            b2row = cb[0:1, 130:162].bitcast(mybir.dt.bfloat16)  # [1, 64]
            ones_row = nc.const_aps.tensor(1.0, (1, 128), mybir.dt.bfloat16)

            # ---- result image: max/max_index write straight into it ----
            # per tile: cols 0:8 top-8 values (f32 bits), 8:16 top-8 indices
            packed = rpool.tile([128, ntiles, 16], I32, tag="packed")

            offs = [sum(schedule[:i]) for i in range(len(schedule))]
            xts, hrs, lps = {}, {}, {}

            def load_chunk(ci, half=None):
                L = schedule[ci]
                if half is None or half == 0:
                    xt = xpool.tile([128, 8, L], F16, tag="xt", name=f"xt{ci}")
                    xts[ci] = xt
                xt = xts[ci]
                ks = slice(0, 8) if half is None else slice(4 * half, 4 * half + 4)
                nk = 8 if half is None else 4
                nc.sync.dma_start(
                    xt[:, ks, :],
                    bass.AP(xT, offs[ci] + (0 if not half else 4 * 128 * nt),
                            [[nt, 128], [128 * nt, nk], [1, L]]))

            def mm1(ci):
                L = schedule[ci]
                xt = xts[ci]
                hr = []
                for m, w_sb in ((0, w1a_sb), (1, w1b_sb)):
                    hp = hpsum.tile([128, L], F32, tag=f"h{m}",
                                    name=f"hp{m}_{ci}", padded_shape=[128, 512])
                    for k in range(8):
                        nc.tensor.matmul(
                            hp[:, :], w_sb[:, k, :], xt[:, k, :],
                            start=(k == 0), stop=(k == 7))
                    hrm = hpool.tile([128, L], F16, tag=f"hr{m}",
                                     name=f"hr{m}_{ci}", padded_shape=[128, 512])
                    nc.scalar.activation(hrm[:, :], hp[:, :], AF.Relu,
                                         bias=b1_sb[:, m:m + 1])
                    hr.append(hrm)
                hrs[ci] = hr

            def mm2(ci):
                L = schedule[ci]
                ns = L // 128
                hr = hrs[ci]
                lp = lpsum.tile([128, ns, NUM_EXPERTS], F32, tag="lg",
                                name=f"lp{ci}",
                                padded_shape=[128, 4, NUM_EXPERTS])
                # in the tail, fold b2 in via a ones-row matmul (27ns on the
                # PE) so the tail-critical DVE chain skips the +b2 add and
                # reads logits straight from PSUM
                fold_b2 = ci >= len(schedule) - 2
                for s in range(ns):
                    if fold_b2:
                        nc.tensor.matmul(lp[:, s, :], ones_row[0:1, :],
                                         b2row[0:1, :], start=True, stop=False)
                    nc.tensor.matmul(lp[:, s, :],
                                     hr[0][:, s * 128:(s + 1) * 128],
                                     w2v[:, 0:64], start=not fold_b2,
                                     stop=False)
                    nc.tensor.matmul(lp[:, s, :],
                                     hr[1][:, s * 128:(s + 1) * 128],
                                     w2v[:, 64:128], start=False, stop=True)
                lps[ci] = lp

            def topk(ci):
                L = schedule[ci]
                ns = L // 128
                t0 = offs[ci] // 128
                lp = lps[ci]
                if ci >= len(schedule) - 2:
                    for s in range(ns):
                        t = t0 + s
                        nc.vector.max(out=packed[:, t, 0:8].bitcast(F32),
                                      in_=lp[:, s, :])
                        nc.vector.max_index(
                            out=packed[:, t, 8:16].bitcast(U32),
                            in_max=packed[:, t, 0:8].bitcast(F32),
                            in_values=lp[:, s, :])
                    return
                lg = lspool.tile([128, ns, NUM_EXPERTS], F32, tag="lsb",
                                 name=f"lg{ci}",
                                 padded_shape=[128, 4, NUM_EXPERTS])
                for s in range(ns):
                    t = t0 + s
                    nc.vector.tensor_add(lg[:, s, :], lp[:, s, :], b2_sb)
                    nc.vector.max(out=packed[:, t, 0:8].bitcast(F32),
                                  in_=lg[:, s, :])
                    nc.vector.max_index(out=packed[:, t, 8:16].bitcast(U32),
                                        in_max=packed[:, t, 0:8].bitcast(F32),
                                        in_values=lg[:, s, :])

            # DMA stream order: w1a, x0 (two halves), w1b, x1, cblob, x2...
            load_w1a = lambda: nc.sync.dma_start(
                w1a_sb[:, :, :],
                bass.AP(w1a, 0, [[8 * 128, 128], [128, 8], [1, 128]]))
            load_w1b = lambda: nc.sync.dma_start(
                w1b_sb[:, :, :],
                bass.AP(w1b, 0, [[8 * 128, 128], [128, 8], [1, 128]]))
            load_cb = lambda: nc.sync.dma_start(
                cb[:, :], bass.AP(cblob, 0, [[162, 128], [1, 162]]))

            # first chunks stream in half-k pieces so the PE can start on
            # k=0..3 while k=4..7 is still in flight
            load_w1a()
            load_chunk(0, half=0)
            load_chunk(0, half=1)
            load_w1b()
            for ci in range(1, n_split):
                load_chunk(ci, half=0)
                load_chunk(ci, half=1)
            load_cb()
            for ci in range(n_split, min(n_split + 1, nchunks)):
                load_chunk(ci)

            t_split = offs[nchunks - split_back] // 128
            for ci in range(nchunks):
                if n_split < ci + 1 < nchunks and ci >= 1:
                    load_chunk(ci + 1)
                mm1(ci)
                if ci > 0:
                    mm2(ci - 1)
                    topk(ci - 1)
                if ci == nchunks - 1:
                    # bulk of the output: its DMA chain overlaps the tail
                    # compute (all x loads are already dispatched on SP)
                    nc.sync.dma_start(
                        bass.AP(out, 0, [[ntiles * 16, 128], [1, t_split * 16]]),
                        packed[:, 0:t_split, :])
            mm2(nchunks - 1)
            topk(nchunks - 1)

            nc.sync.dma_start(
                bass.AP(out, t_split * 16,
                        [[ntiles * 16, 128], [1, (ntiles - t_split) * 16]]),
                packed[:, t_split:ntiles, :])

    _split_excess_waits(nc)
    _strip_trailing_barrier(nc)
    _strip_unused_const_memsets(nc)
    return nc


def shard_inputs(x, w1, b1, w2, b2, n_cores=N_CORES):
    nt = x.shape[0] // n_cores
    w1T = np.ascontiguousarray(w1.T).astype(np.float16)        # [1024, 256]
    w1r = w1T.reshape(8, 128, HIDDEN_DIM)                      # [k, p, h]
    w1ai = np.ascontiguousarray(
        w1r[:, :, 0:128].transpose(1, 0, 2).reshape(128, 8 * 128))
    w1bi = np.ascontiguousarray(
        w1r[:, :, 128:256].transpose(1, 0, 2).reshape(128, 8 * 128))
    w2T = np.ascontiguousarray(w2.T).astype(np.float16)        # [256, 64]
    w2i = np.ascontiguousarray(
        w2T.reshape(2, 128, NUM_EXPERTS).transpose(1, 0, 2)
        .reshape(128, 2 * NUM_EXPERTS))                        # [128, 128] f16
    b1i = np.ascontiguousarray(b1.reshape(2, 128).T.astype(np.float32))
    cblob = np.zeros((128, 162), np.int32)
    cblob[:, 0:2] = b1i.view(np.int32)
    cblob[:, 2:66] = w2i.view(np.int32)
    cblob[:, 66:130] = np.broadcast_to(
        b2.astype(np.float32).view(np.int32), (128, 64))
    # b2 as bf16 (round-to-nearest-even) for the tail ones-row matmul
    b2u = b2.astype(np.float32).view(np.uint32)
    b2bf = ((b2u + 0x7FFF + ((b2u >> 16) & 1)) >> 16).astype(np.uint16)
    cblob[0, 130:162] = b2bf.view(np.int32)
    xT = np.ascontiguousarray(x.T.astype(np.float16))          # [1024, N]
    return [
        {"xT": np.ascontiguousarray(xT[:, c * nt:(c + 1) * nt]),
         "w1a": w1ai, "w1b": w1bi, "cblob": cblob}
        for c in range(n_cores)
    ]


def unshard_outputs(results, nt=NT):
    ntiles = nt // 128
    idxs, maxes = [], []
    for res in results:
        packed = res["out"].reshape(128, ntiles, 16)
        m = np.ascontiguousarray(packed[:, :, 0:3]).view(np.float32)
        i = packed[:, :, 8:10]
        maxes.append(m.transpose(1, 0, 2).reshape(nt, 3))
        idxs.append(i.transpose(1, 0, 2).reshape(nt, 2).astype(np.int32))
    return np.concatenate(idxs), np.concatenate(maxes)


def host_gates(maxes):
    """softmax over the top-2 logits, from the exported top-3 values."""
    d = (maxes[:, 1] - maxes[:, 0]).astype(np.float32)
    e = np.exp(d)
    g1 = 1.0 / (1.0 + e)
    return np.stack([g1, e * g1], axis=1).astype(np.float32)


def margin_fixup(idx, gates, maxes, x, w1, b1, w2, b2, tau=FIXUP_TAU):
    """Exactly recompute tokens whose device top-3 margins are below tau."""
    margin = np.minimum(maxes[:, 0] - maxes[:, 1], maxes[:, 1] - maxes[:, 2])
    bad = np.where(margin < tau)[0]
    if len(bad) == 0:
        return idx, gates, bad
    xb = x[bad].astype(np.float64)
    h = np.maximum(xb @ w1.astype(np.float64).T + b1.astype(np.float64), 0)
    logits = h @ w2.astype(np.float64).T + b2.astype(np.float64)
    order = np.argsort(-logits, axis=1)[:, :2]
    m = np.take_along_axis(logits, order, axis=1)
    e = np.exp(m - m[:, :1])
    g = (e / e.sum(axis=1, keepdims=True)).astype(np.float32)
    idx = idx.copy(); gates = gates.copy()
    idx[bad] = order.astype(np.int32)
    gates[bad] = g
    return idx, gates, bad


_NC_CACHE = None


def _get_nc():
    global _NC_CACHE
    if _NC_CACHE is None:
        _NC_CACHE = build_kernel()
    return _NC_CACHE


def run_on_device(x, w1, b1, w2, b2, **spmd_kwargs):
    """Run the Bass kernel on the 8 cores; returns (idx, maxes) plus
    the raw BassKernelResults (for profiling)."""
    in_maps = shard_inputs(x, w1, b1, w2, b2)
    res = run_bass_kernel_spmd(_get_nc(), in_maps, list(range(N_CORES)),
                               **spmd_kwargs)
    idx, maxes = unshard_outputs(res.results)
    return idx, maxes, res


def kernel(x, w1, b1, w2, b2):
    x = np.asarray(x, dtype=np.float32)
    w1 = np.asarray(w1, dtype=np.float32)
    b1 = np.asarray(b1, dtype=np.float32)
    w2 = np.asarray(w2, dtype=np.float32)
    b2 = np.asarray(b2, dtype=np.float32)
    idx, maxes, _ = run_on_device(x, w1, b1, w2, b2)
    gates = host_gates(maxes)
    idx, gates, _ = margin_fixup(idx, gates, maxes, x, w1, b1, w2, b2)
    return idx.astype(np.int32), gates.astype(np.float32)
